# revision 9
# baseline (speedup 1.0000x reference)
"""DenseGRUODE Trainium2 Bass kernel.

Reference computation (per step t, Euler GRU-ODE):
    hx  = [h, x_t]                      # [B, 192]
    r   = sigmoid(hx @ W_hr + b_hr)     # [B, 128]
    z   = sigmoid(hx @ W_hz + b_hz)
    u   = tanh([r*h, x_t] @ W_hh + b_hh)
    h'  = h + (1-z)*(u-h)*dt
Output: hs transposed to [B, T, 128].

Device mapping (per core, data-parallel over batch, BC = 256/8 = 32):
  * Transposed activation layout: h kept as hT [128 feat partitions, BC free].
    All matmuls contract over the partition dim, so the gate pre-activations
    are computed as  aT[128, BC] = W_part.T @ hT  (lhsT = W rows as stored).
  * x contributions + biases are precomputed per 16-step chunk with one
    [65]x[128]x[512] matmul per gate into a PSUM bank (ones row folds the
    bias in); the per-step K=128 h-part matmul accumulates into the bank
    slice for that step.
  * z weights are pre-negated on the host so one Sigmoid yields s = 1-z.
    h' = (1 - dt*s)*h + (dt*s)*u  computed with DVE tensor ops.
  * Output transpose (feat-major -> batch-major) via DVE 32x32 block
    transpose every 4 steps, then one DMA of [128,128] to DRAM.
"""

import numpy as np

T = 1000
B = 256
NCORES = 8
BC = B // NCORES  # 32
DIM_IN = 64
DIM_OUT = 128
KX = DIM_IN + 1  # x rows + ones row (bias)
DT = 0.05
CHUNK = 16  # steps per PSUM bank (16*32 = 512 fp32 = one bank)
TGROUP = 8  # steps per output transpose/DMA group


def _build_nc(t_steps=T):
    import concourse.bacc as bacc
    import concourse.mybir as mybir
    import concourse.tile as tile
    from contextlib import ExitStack

    f32 = mybir.dt.float32
    AF = mybir.ActivationFunctionType
    ALU = mybir.AluOpType

    nc = bacc.Bacc("TRN2", target_bir_lowering=False, debug=False)

    xa = nc.dram_tensor("xa", [KX, t_steps * BC], f32, kind="ExternalInput")
    wrh_d = nc.dram_tensor("wrh", [DIM_OUT, DIM_OUT], f32, kind="ExternalInput")
    wzh_d = nc.dram_tensor("wzh", [DIM_OUT, DIM_OUT], f32, kind="ExternalInput")
    whh_d = nc.dram_tensor("whh", [DIM_OUT, DIM_OUT], f32, kind="ExternalInput")
    wrx_d = nc.dram_tensor("wrx", [KX, DIM_OUT], f32, kind="ExternalInput")
    wzx_d = nc.dram_tensor("wzx", [KX, DIM_OUT], f32, kind="ExternalInput")
    whx_d = nc.dram_tensor("whx", [KX, DIM_OUT], f32, kind="ExternalInput")
    h0_d = nc.dram_tensor("h0", [DIM_OUT, BC], f32, kind="ExternalInput")
    out_d = nc.dram_tensor("out", [BC, t_steps, DIM_OUT], f32, kind="ExternalOutput")

    nchunks = (t_steps + CHUNK - 1) // CHUNK

    def csize(c):
        return min(CHUNK, t_steps - c * CHUNK)

    with tile.TileContext(nc) as tc, ExitStack() as ctx:
        consts = ctx.enter_context(tc.tile_pool(name="consts", bufs=1))
        xpool = ctx.enter_context(tc.tile_pool(name="xchunk", bufs=2))
        ppr = ctx.enter_context(tc.tile_pool(name="psr", bufs=2, space="PSUM"))
        ppz = ctx.enter_context(tc.tile_pool(name="psz", bufs=2, space="PSUM"))
        pph = ctx.enter_context(tc.tile_pool(name="psh", bufs=2, space="PSUM"))
        hpool = ctx.enter_context(tc.tile_pool(name="hbuf", bufs=3))
        spool = ctx.enter_context(tc.tile_pool(name="stage", bufs=3))
        work = ctx.enter_context(tc.tile_pool(name="work", bufs=3))

        def load_const(dram, shape, cname):
            ctile = consts.tile(shape, f32, tag=cname, name=cname + "_s")
            nc.sync.dma_start(ctile[:, :], dram.ap())
            return ctile

        wrh = load_const(wrh_d, [DIM_OUT, DIM_OUT], "wrh")
        wzh = load_const(wzh_d, [DIM_OUT, DIM_OUT], "wzh")
        whh = load_const(whh_d, [DIM_OUT, DIM_OUT], "whh")
        wrx = load_const(wrx_d, [KX, DIM_OUT], "wrx")
        wzx = load_const(wzx_d, [KX, DIM_OUT], "wzx")
        whx = load_const(whx_d, [KX, DIM_OUT], "whx")
        h0 = load_const(h0_d, [DIM_OUT, BC], "h0")

        gates = [("r", ppr, wrx), ("z", ppz, wzx), ("h", pph, whx)]
        psum_tiles = {}

        def emit_prologue(c):
            n = csize(c) * BC
            lo = c * CHUNK * BC
            xt = xpool.tile([KX, CHUNK * BC], f32, tag="x", name=f"x_{c}")
            nc.sync.dma_start(xt[:, :n], xa[:, lo : lo + n])
            for gname, pp, wx in gates:
                ps = pp.tile([DIM_OUT, CHUNK * BC], f32, tag=gname, name=f"ps{gname}_{c}")
                nc.tensor.matmul(ps[:, :n], wx[:, :], xt[:, :n], start=True, stop=True)
                psum_tiles[(c, gname)] = ps

        emit_prologue(0)
        h_prev = h0
        hbuf = None

        for t in range(t_steps):
            c, s = divmod(t, CHUNK)
            if s == 4 and c + 1 < nchunks:
                emit_prologue(c + 1)
            last = s == csize(c) - 1
            sl = slice(s * BC, (s + 1) * BC)
            ps_r = psum_tiles[(c, "r")]
            ps_z = psum_tiles[(c, "z")]
            ps_h = psum_tiles[(c, "h")]
            if t % TGROUP == 0:
                hbuf = hpool.tile([DIM_OUT, TGROUP * BC], f32, tag="h", name=f"hb_{t}")

            nc.tensor.matmul(
                ps_r[:, sl], wrh[:, :], h_prev, start=False, stop=True,
                skip_group_check=True,
            )
            nc.tensor.matmul(
                ps_z[:, sl], wzh[:, :], h_prev, start=False, stop=True,
                skip_group_check=True,
            )

            r = work.tile([DIM_OUT, BC], f32, tag="r", name=f"r_{t}")
            nc.scalar.activation(r[:, :], ps_r[:, sl], AF.Sigmoid)
            sz = work.tile([DIM_OUT, BC], f32, tag="sz", name=f"sz_{t}")
            nc.scalar.activation(sz[:, :], ps_z[:, sl], AF.Sigmoid)

            rh = work.tile([DIM_OUT, BC], f32, tag="rh", name=f"rh_{t}")
            nc.vector.tensor_mul(rh[:, :], r[:, :], h_prev)
            nc.tensor.matmul(
                ps_h[:, sl], whh[:, :], rh[:, :], start=False, stop=True,
                skip_group_check=True,
            )
            u = work.tile([DIM_OUT, BC], f32, tag="u", name=f"u_{t}")
            nc.scalar.activation(u[:, :], ps_h[:, sl], AF.Tanh)

            # g = dt*s ; q = 1 - dt*s ; h' = q*h + g*u
            g = work.tile([DIM_OUT, BC], f32, tag="g", name=f"g_{t}")
            nc.vector.tensor_scalar_mul(g[:, :], sz[:, :], DT)
            q = work.tile([DIM_OUT, BC], f32, tag="q", name=f"q_{t}")
            nc.vector.tensor_scalar(q[:, :], sz[:, :], -DT, 1.0, ALU.mult, ALU.add)
            pre = work.tile([DIM_OUT, BC], f32, tag="pre", name=f"pre_{t}")
            nc.vector.tensor_mul(pre[:, :], q[:, :], h_prev)
            t1 = work.tile([DIM_OUT, BC], f32, tag="t1", name=f"t1_{t}")
            nc.vector.tensor_mul(t1[:, :], g[:, :], u[:, :])
            hnew = hbuf[:, (t % TGROUP) * BC : (t % TGROUP + 1) * BC]
            nc.vector.tensor_add(hnew, t1[:, :], pre[:, :])
            h_prev = hnew

            if t % TGROUP == TGROUP - 1:
                stg = spool.tile([DIM_OUT, TGROUP * BC], f32, tag="stg", name=f"st_{t}")
                nc.vector.transpose(stg[:, :], hbuf[:, :])
                # stg[32i+b, 32s+c] = h_{t0+s}[32i+c, b]; DMA one feature
                # block i at a time (DMA APs are limited to 3 dims).
                for i in range(DIM_OUT // 32):
                    dst = out_d.ap()[
                        0:BC, t - (TGROUP - 1) : t + 1, 32 * i : 32 * (i + 1)
                    ]
                    nc.sync.dma_start(dst, stg[32 * i : 32 * (i + 1), :])

    nc.compile()
    return nc


def _host_prep(X, W_hr, b_hr, W_hz, b_hz, W_hh, b_hh, h0, t_steps=T):
    f = np.float32
    X = np.asarray(X, f)[:t_steps]
    W_hr, W_hz, W_hh = (np.asarray(w, f) for w in (W_hr, W_hz, W_hh))
    b_hr, b_hz, b_hh = (np.asarray(b, f) for b in (b_hr, b_hz, b_hh))
    h0 = np.asarray(h0, f).reshape(1, DIM_OUT)

    XT = np.ascontiguousarray(np.transpose(X, (2, 0, 1)))  # [64, T, B]
    wrx = np.ascontiguousarray(np.vstack([W_hr[DIM_OUT:], b_hr[None, :]]))
    wzx = np.ascontiguousarray(-np.vstack([W_hz[DIM_OUT:], b_hz[None, :]]))
    whx = np.ascontiguousarray(np.vstack([W_hh[DIM_OUT:], b_hh[None, :]]))
    wrh = np.ascontiguousarray(W_hr[:DIM_OUT])
    wzh = np.ascontiguousarray(-W_hz[:DIM_OUT])
    whh = np.ascontiguousarray(W_hh[:DIM_OUT])
    h0T = np.ascontiguousarray(np.broadcast_to(h0.T, (DIM_OUT, BC)))

    in_maps = []
    for ci in range(NCORES):
        xc = XT[:, :, ci * BC : (ci + 1) * BC].reshape(DIM_IN, t_steps * BC)
        xa = np.ascontiguousarray(
            np.vstack([xc, np.ones((1, t_steps * BC), f)])
        )
        in_maps.append(
            {
                "xa": xa,
                "wrh": wrh,
                "wzh": wzh,
                "whh": whh,
                "wrx": wrx,
                "wzx": wzx,
                "whx": whx,
                "h0": h0T,
            }
        )
    return in_maps


def run(inputs, trace=False, t_steps=T, tmpdir=None):
    from concourse import bass_utils

    in_maps = _host_prep(**inputs, t_steps=t_steps)
    nc = _build_nc(t_steps)
    res = bass_utils.run_bass_kernel_spmd(
        nc, in_maps, core_ids=list(range(NCORES)), trace=trace, tmpdir=tmpdir
    )
    out = np.concatenate([res.results[i]["out"] for i in range(NCORES)], axis=0)
    return out, res


def kernel(**inputs) -> np.ndarray:
    out, _ = run(inputs, trace=False)
    return out


# revision 10
# speedup vs baseline: 1.7688x; 1.7688x over previous
"""DenseGRUODE Trainium2 Bass kernel.

Reference computation (per step t, Euler GRU-ODE):
    hx  = [h, x_t]                      # [B, 192]
    r   = sigmoid(hx @ W_hr + b_hr)     # [B, 128]
    z   = sigmoid(hx @ W_hz + b_hz)
    u   = tanh([r*h, x_t] @ W_hh + b_hh)
    h'  = h + (1-z)*(u-h)*dt
Output: hs transposed to [B, T, 128].

Device mapping (per core, data-parallel over batch, BC = 256/8 = 32):
  * Transposed activation layout: h kept as hT [128 feat partitions, BC free].
    Matmuls contract over the partition dim:  aT[128,BC] = W_part.T @ hT.
  * fp16 matmuls (4x faster than fp32 on the PE, which needs 2 passes at
    4 cyc/row for fp32).  Gate h-weights are fp16; x-part weights are
    SPLIT into hi+lo fp16 matrices to cancel systematic quantization.
    Everything else (PSUM accumulate, activations, state h) stays fp32;
    measured end-to-end error vs f64 reference: ~2e-4.
  * x contributions + biases are precomputed per 16-step chunk with
    [65]x[128]x[512] fp16 matmuls per gate into a PSUM bank (ones row
    folds the bias); per-step h-part matmuls accumulate into the bank
    slice for that step.
  * z weights are pre-negated so one Sigmoid yields s = 1-z directly.
  * Critical-path split:  h' = pre + t1 with pre = (1-dt*s)*h (ready
    early) and t1 = dt*s*u (ready late).  The next step's r/z matmuls
    consume pre_f16 and t1_f16 separately (PSUM adds them), so the
    fp32 h' reconstruction is OFF the serial critical path.
  * Output transpose (feat-major -> batch-major) via DVE 32x32 block
    transpose every 8 steps, then 4 DMAs (one per 32-feature block).
"""

import numpy as np

T = 1000
B = 256
NCORES = 8
BC = B // NCORES  # 32
DIM_IN = 64
DIM_OUT = 128
KX = DIM_IN + 1  # x rows + ones row (bias)
DT = 0.05
CHUNK = 16  # steps per PSUM bank (16*32 = 512 fp32 = one bank)
TGROUP = 8  # steps per output transpose/DMA group


def _build_nc(t_steps=T):
    import concourse.bacc as bacc
    import concourse.mybir as mybir
    import concourse.tile as tile
    from contextlib import ExitStack

    f32 = mybir.dt.float32
    f16 = mybir.dt.float16
    AF = mybir.ActivationFunctionType
    ALU = mybir.AluOpType

    nc = bacc.Bacc("TRN2", target_bir_lowering=False, debug=False)

    xa = nc.dram_tensor("xa", [KX, t_steps * BC], f16, kind="ExternalInput")
    wrh_d = nc.dram_tensor("wrh", [DIM_OUT, DIM_OUT], f16, kind="ExternalInput")
    wzh_d = nc.dram_tensor("wzh", [DIM_OUT, DIM_OUT], f16, kind="ExternalInput")
    whh_d = nc.dram_tensor("whh", [DIM_OUT, DIM_OUT], f16, kind="ExternalInput")
    # x-part weights, hi+lo fp16 split, bias folded in via the ones row
    wx_d = {}
    for g in ("r", "z", "h"):
        for p in ("hi", "lo"):
            wx_d[(g, p)] = nc.dram_tensor(
                f"w{g}x_{p}", [KX, DIM_OUT], f16, kind="ExternalInput"
            )
    h0_d = nc.dram_tensor("h0", [DIM_OUT, BC], f32, kind="ExternalInput")
    out_d = nc.dram_tensor("out", [BC, t_steps, DIM_OUT], f32, kind="ExternalOutput")

    nchunks = (t_steps + CHUNK - 1) // CHUNK

    def csize(c):
        return min(CHUNK, t_steps - c * CHUNK)

    with tile.TileContext(nc) as tc, ExitStack() as ctx:
        consts = ctx.enter_context(tc.tile_pool(name="consts", bufs=1))
        xpool = ctx.enter_context(tc.tile_pool(name="xchunk", bufs=2))
        ppr = ctx.enter_context(tc.tile_pool(name="psr", bufs=2, space="PSUM"))
        ppz = ctx.enter_context(tc.tile_pool(name="psz", bufs=2, space="PSUM"))
        pph = ctx.enter_context(tc.tile_pool(name="psh", bufs=2, space="PSUM"))
        hpool = ctx.enter_context(tc.tile_pool(name="hbuf", bufs=3))
        spool = ctx.enter_context(tc.tile_pool(name="stage", bufs=3))
        work = ctx.enter_context(tc.tile_pool(name="work", bufs=3))

        def load_const(dram, shape, cname, dt_):
            ctile = consts.tile(shape, dt_, tag=cname, name=cname + "_s")
            nc.sync.dma_start(ctile[:, :], dram.ap())
            return ctile

        wrh = load_const(wrh_d, [DIM_OUT, DIM_OUT], "wrh", f16)
        wzh = load_const(wzh_d, [DIM_OUT, DIM_OUT], "wzh", f16)
        whh = load_const(whh_d, [DIM_OUT, DIM_OUT], "whh", f16)
        wx = {
            k: load_const(d, [KX, DIM_OUT], f"wx{k[0]}{k[1]}", f16)
            for k, d in wx_d.items()
        }
        h0 = load_const(h0_d, [DIM_OUT, BC], "h0", f32)

        # initial state: h = h0 (f32); pre16 = f16(h0); no t1 yet
        pre16 = work.tile([DIM_OUT, BC], f16, tag="pre16", name="pre16_init")
        nc.vector.tensor_copy(pre16[:, :], h0[:, :])

        gates = [("r", ppr), ("z", ppz), ("h", pph)]
        psum_tiles = {}

        def emit_prologue(c):
            n = csize(c) * BC
            lo = c * CHUNK * BC
            xt = xpool.tile([KX, CHUNK * BC], f16, tag="x", name=f"x_{c}")
            nc.sync.dma_start(xt[:, :n], xa[:, lo : lo + n])
            for gname, pp in gates:
                ps = pp.tile([DIM_OUT, CHUNK * BC], f32, tag=gname, name=f"ps{gname}_{c}")
                nc.tensor.matmul(
                    ps[:, :n], wx[(gname, "hi")][:, :], xt[:, :n], start=True, stop=True
                )
                nc.tensor.matmul(
                    ps[:, :n], wx[(gname, "lo")][:, :], xt[:, :n],
                    start=False, stop=True, skip_group_check=True,
                )
                psum_tiles[(c, gname)] = ps

        emit_prologue(0)
        h_prev = h0
        t116 = None
        hbuf = None

        def acc_mm(ps, sl, w, rhs):
            nc.tensor.matmul(
                ps[:, sl], w[:, :], rhs[:, :], start=False, stop=True,
                skip_group_check=True,
            )

        for t in range(t_steps):
            c, s = divmod(t, CHUNK)
            if s == 4 and c + 1 < nchunks:
                emit_prologue(c + 1)
            sl = slice(s * BC, (s + 1) * BC)
            ps_r = psum_tiles[(c, "r")]
            ps_z = psum_tiles[(c, "z")]
            ps_h = psum_tiles[(c, "h")]
            if t % TGROUP == 0:
                hbuf = hpool.tile([DIM_OUT, TGROUP * BC], f32, tag="h", name=f"hb_{t}")

            # gate pre-activations: psum slice = xpart (+bias) + W@pre + W@t1
            acc_mm(ps_r, sl, wrh, pre16)
            acc_mm(ps_z, sl, wzh, pre16)
            if t116 is not None:
                acc_mm(ps_r, sl, wrh, t116)
                acc_mm(ps_z, sl, wzh, t116)

            r = work.tile([DIM_OUT, BC], f32, tag="r", name=f"r_{t}")
            nc.scalar.activation(r[:, :], ps_r[:, sl], AF.Sigmoid)
            sz = work.tile([DIM_OUT, BC], f32, tag="sz", name=f"sz_{t}")
            nc.scalar.activation(sz[:, :], ps_z[:, sl], AF.Sigmoid)

            rh16 = work.tile([DIM_OUT, BC], f16, tag="rh16", name=f"rh_{t}")
            nc.vector.tensor_mul(rh16[:, :], r[:, :], h_prev)
            acc_mm(ps_h, sl, whh, rh16)
            u = work.tile([DIM_OUT, BC], f32, tag="u", name=f"u_{t}")
            nc.scalar.activation(u[:, :], ps_h[:, sl], AF.Tanh)

            # t1 = dt*u*s  (fp16, feeds next step's matmuls; ON critical path)
            t116 = work.tile([DIM_OUT, BC], f16, tag="t116", name=f"t1_{t}")
            nc.vector.scalar_tensor_tensor(
                t116[:, :], u[:, :], DT, sz[:, :], ALU.mult, ALU.mult
            )
            # q = 1 - dt*s ; pre = q*h (f32 + f16 copy); h' = pre + t1 (f32)
            q = work.tile([DIM_OUT, BC], f32, tag="q", name=f"q_{t}")
            nc.vector.tensor_scalar(q[:, :], sz[:, :], -DT, 1.0, ALU.mult, ALU.add)
            pre32 = work.tile([DIM_OUT, BC], f32, tag="pre32", name=f"pre32_{t}")
            nc.vector.tensor_mul(pre32[:, :], q[:, :], h_prev)
            pre16 = work.tile([DIM_OUT, BC], f16, tag="pre16", name=f"pre16_{t}")
            nc.vector.tensor_copy(pre16[:, :], pre32[:, :])
            hnew = hbuf[:, (t % TGROUP) * BC : (t % TGROUP + 1) * BC]
            nc.vector.tensor_add(hnew, pre32[:, :], t116[:, :])
            h_prev = hnew

            if t % TGROUP == TGROUP - 1:
                stg = spool.tile([DIM_OUT, TGROUP * BC], f32, tag="stg", name=f"st_{t}")
                nc.vector.transpose(stg[:, :], hbuf[:, :])
                # stg[32i+b, 32s+c] = h_{t0+s}[32i+c, b]; DMA one feature
                # block i at a time (DMA APs are limited to 3 dims).
                for i in range(DIM_OUT // 32):
                    dst = out_d.ap()[
                        0:BC, t - (TGROUP - 1) : t + 1, 32 * i : 32 * (i + 1)
                    ]
                    nc.sync.dma_start(dst, stg[32 * i : 32 * (i + 1), :])

    nc.compile()
    return nc


def _host_prep(X, W_hr, b_hr, W_hz, b_hz, W_hh, b_hh, h0, t_steps=T):
    f = np.float32
    X = np.asarray(X, f)[:t_steps]
    W_hr, W_hz, W_hh = (np.asarray(w, f) for w in (W_hr, W_hz, W_hh))
    b_hr, b_hz, b_hh = (np.asarray(b, f) for b in (b_hr, b_hz, b_hh))
    h0 = np.asarray(h0, f).reshape(1, DIM_OUT)

    XT = np.ascontiguousarray(np.transpose(X, (2, 0, 1)))  # [64, T, B]
    weights = {
        "wrh": W_hr[:DIM_OUT].astype(np.float16),
        "wzh": (-W_hz[:DIM_OUT]).astype(np.float16),
        "whh": W_hh[:DIM_OUT].astype(np.float16),
    }
    for g, W, b, sgn in (
        ("r", W_hr, b_hr, 1.0),
        ("z", W_hz, b_hz, -1.0),
        ("h", W_hh, b_hh, 1.0),
    ):
        wxb = sgn * np.vstack([W[DIM_OUT:], b[None, :]])  # [65, 128] f32
        hi = wxb.astype(np.float16)
        lo = (wxb - hi.astype(f)).astype(np.float16)
        weights[f"w{g}x_hi"] = np.ascontiguousarray(hi)
        weights[f"w{g}x_lo"] = np.ascontiguousarray(lo)
    weights = {k: np.ascontiguousarray(v) for k, v in weights.items()}
    h0T = np.ascontiguousarray(np.broadcast_to(h0.T, (DIM_OUT, BC)))

    in_maps = []
    for ci in range(NCORES):
        xc = XT[:, :, ci * BC : (ci + 1) * BC].reshape(DIM_IN, t_steps * BC)
        xa = np.ascontiguousarray(
            np.vstack([xc, np.ones((1, t_steps * BC), f)]).astype(np.float16)
        )
        m = {"xa": xa, "h0": h0T}
        m.update(weights)
        in_maps.append(m)
    return in_maps


def run(inputs, trace=False, t_steps=T, tmpdir=None):
    from concourse import bass_utils

    in_maps = _host_prep(**inputs, t_steps=t_steps)
    nc = _build_nc(t_steps)
    res = bass_utils.run_bass_kernel_spmd(
        nc, in_maps, core_ids=list(range(NCORES)), trace=trace, tmpdir=tmpdir
    )
    out = np.concatenate([res.results[i]["out"] for i in range(NCORES)], axis=0)
    return out, res


def kernel(**inputs) -> np.ndarray:
    out, _ = run(inputs, trace=False)
    return out


# revision 13
# speedup vs baseline: 1.7710x; 1.0013x over previous
"""DenseGRUODE Trainium2 Bass kernel.

Reference computation (per step t, Euler GRU-ODE):
    hx  = [h, x_t]                      # [B, 192]
    r   = sigmoid(hx @ W_hr + b_hr)     # [B, 128]
    z   = sigmoid(hx @ W_hz + b_hz)
    u   = tanh([r*h, x_t] @ W_hh + b_hh)
    h'  = h + (1-z)*(u-h)*dt
Output: hs transposed to [B, T, 128].

Device mapping (per core, data-parallel over batch, BC = 256/8 = 32):
  * Transposed activation layout: h kept as hT [128 feat partitions, BC free].
    Matmuls contract over the partition dim:  aT[128,BC] = W_part.T @ hT.
  * fp16 matmuls (4x faster than fp32 on the PE, which needs 2 passes at
    4 cyc/row for fp32).  Gate h-weights are fp16; x-part weights are
    SPLIT into hi+lo fp16 matrices to cancel systematic quantization.
    Everything else (PSUM accumulate, activations, state h) stays fp32;
    measured end-to-end error vs f64 reference: ~2e-4.
  * x contributions + biases are precomputed per 16-step chunk with
    [65]x[128]x[512] fp16 matmuls per gate into a PSUM bank (ones row
    folds the bias); per-step h-part matmuls accumulate into the bank
    slice for that step.
  * z weights are pre-negated so one Sigmoid yields s = 1-z directly.
  * Critical-path split:  h' = pre + t1 with pre = (1-dt*s)*h (ready
    early) and t1 = dt*s*u (ready late).  The next step's r/z matmuls
    consume pre_f16 and t1_f16 separately (PSUM adds them), so the
    fp32 h' reconstruction is OFF the serial critical path.
  * Output transpose (feat-major -> batch-major) via DVE 32x32 block
    transpose every 8 steps, then 4 DMAs (one per 32-feature block).
"""

import numpy as np

T = 1000
B = 256
NCORES = 8
BC = B // NCORES  # 32
DIM_IN = 64
DIM_OUT = 128
KX = DIM_IN + 1  # x rows + ones row (bias)
DT = 0.05
CHUNK = 16  # steps per PSUM bank (16*32 = 512 fp32 = one bank)
TGROUP = 8  # steps per output transpose/DMA group


def _build_nc(t_steps=T):
    import concourse.bacc as bacc
    import concourse.mybir as mybir
    import concourse.tile as tile
    from contextlib import ExitStack

    f32 = mybir.dt.float32
    f16 = mybir.dt.float16
    AF = mybir.ActivationFunctionType
    ALU = mybir.AluOpType

    nc = bacc.Bacc("TRN2", target_bir_lowering=False, debug=False)

    xa = nc.dram_tensor("xa", [KX, t_steps * BC], f16, kind="ExternalInput")
    wrh_d = nc.dram_tensor("wrh", [DIM_OUT, DIM_OUT], f16, kind="ExternalInput")
    wzh_d = nc.dram_tensor("wzh", [DIM_OUT, DIM_OUT], f16, kind="ExternalInput")
    whh_d = nc.dram_tensor("whh", [DIM_OUT, DIM_OUT], f16, kind="ExternalInput")
    # x-part weights, hi+lo fp16 split, bias folded in via the ones row
    wx_d = {}
    for g in ("r", "z", "h"):
        for p in ("hi", "lo"):
            wx_d[(g, p)] = nc.dram_tensor(
                f"w{g}x_{p}", [KX, DIM_OUT], f16, kind="ExternalInput"
            )
    h0_d = nc.dram_tensor("h0", [DIM_OUT, BC], f32, kind="ExternalInput")
    out_d = nc.dram_tensor("out", [BC, t_steps, DIM_OUT], f32, kind="ExternalOutput")

    nchunks = (t_steps + CHUNK - 1) // CHUNK

    def csize(c):
        return min(CHUNK, t_steps - c * CHUNK)

    with tile.TileContext(nc) as tc, ExitStack() as ctx:
        consts = ctx.enter_context(tc.tile_pool(name="consts", bufs=1))
        xpool = ctx.enter_context(tc.tile_pool(name="xchunk", bufs=2))
        ppr = ctx.enter_context(tc.tile_pool(name="psr", bufs=2, space="PSUM"))
        ppz = ctx.enter_context(tc.tile_pool(name="psz", bufs=2, space="PSUM"))
        pph = ctx.enter_context(tc.tile_pool(name="psh", bufs=2, space="PSUM"))
        hpool = ctx.enter_context(tc.tile_pool(name="hbuf", bufs=3))
        spool = ctx.enter_context(tc.tile_pool(name="stage", bufs=3))
        work = ctx.enter_context(tc.tile_pool(name="work", bufs=3))

        def load_const(dram, shape, cname, dt_):
            ctile = consts.tile(shape, dt_, tag=cname, name=cname + "_s")
            nc.sync.dma_start(ctile[:, :], dram.ap())
            return ctile

        wrh = load_const(wrh_d, [DIM_OUT, DIM_OUT], "wrh", f16)
        wzh = load_const(wzh_d, [DIM_OUT, DIM_OUT], "wzh", f16)
        whh = load_const(whh_d, [DIM_OUT, DIM_OUT], "whh", f16)
        wx = {
            k: load_const(d, [KX, DIM_OUT], f"wx{k[0]}{k[1]}", f16)
            for k, d in wx_d.items()
        }
        h0 = load_const(h0_d, [DIM_OUT, BC], "h0", f32)

        # initial state: h = h0 (f32); pre16 = f16(h0); no t1 yet
        pre16 = work.tile([DIM_OUT, BC], f16, tag="pre16", name="pre16_init")
        nc.vector.tensor_copy(pre16[:, :], h0[:, :])

        gates = [("r", ppr), ("z", ppz), ("h", pph)]
        psum_tiles = {}

        def emit_chunk_dma(c):
            n = csize(c) * BC
            lo = c * CHUNK * BC
            xt = xpool.tile([KX, CHUNK * BC], f16, tag="x", name=f"x_{c}")
            nc.sync.dma_start(xt[:, :n], xa[:, lo : lo + n])
            return xt

        def emit_chunk_mm(c, xt, j):
            # one of the 6 x-part matmuls (gate x hi/lo); spread across
            # steps so they don't pile up in the PE FIFO ahead of the
            # latency-critical per-step matmuls
            n = csize(c) * BC
            gname, pp = gates[j // 2]
            part = ("hi", "lo")[j % 2]
            if j % 2 == 0:
                ps = pp.tile([DIM_OUT, CHUNK * BC], f32, tag=gname, name=f"ps{gname}_{c}")
                psum_tiles[(c, gname)] = ps
                nc.tensor.matmul(
                    ps[:, :n], wx[(gname, part)][:, :], xt[:, :n], start=True, stop=True
                )
            else:
                ps = psum_tiles[(c, gname)]
                nc.tensor.matmul(
                    ps[:, :n], wx[(gname, part)][:, :], xt[:, :n],
                    start=False, stop=True, skip_group_check=True,
                )

        xt0 = emit_chunk_dma(0)
        for j in range(6):
            emit_chunk_mm(0, xt0, j)
        next_xt = None
        h_prev = h0
        t116 = None
        hbuf = None

        def acc_mm(ps, sl, w, rhs):
            nc.tensor.matmul(
                ps[:, sl], w[:, :], rhs[:, :], start=False, stop=True,
                skip_group_check=True,
            )

        for t in range(t_steps):
            c, s = divmod(t, CHUNK)
            if s == 2 and c + 1 < nchunks:
                next_xt = emit_chunk_dma(c + 1)
            if 4 <= s < 10 and c + 1 < nchunks:
                emit_chunk_mm(c + 1, next_xt, s - 4)
            sl = slice(s * BC, (s + 1) * BC)
            ps_r = psum_tiles[(c, "r")]
            ps_z = psum_tiles[(c, "z")]
            ps_h = psum_tiles[(c, "h")]
            if t % TGROUP == 0:
                hbuf = hpool.tile([DIM_OUT, TGROUP * BC], f32, tag="h", name=f"hb_{t}")

            # gate pre-activations: psum slice = xpart (+bias) + W@pre + W@t1
            acc_mm(ps_r, sl, wrh, pre16)
            acc_mm(ps_z, sl, wzh, pre16)
            if t116 is not None:
                acc_mm(ps_r, sl, wrh, t116)
                acc_mm(ps_z, sl, wzh, t116)

            r = work.tile([DIM_OUT, BC], f32, tag="r", name=f"r_{t}")
            nc.scalar.activation(r[:, :], ps_r[:, sl], AF.Sigmoid)
            sz = work.tile([DIM_OUT, BC], f16, tag="sz", name=f"sz_{t}")
            nc.scalar.activation(sz[:, :], ps_z[:, sl], AF.Sigmoid)

            rh16 = work.tile([DIM_OUT, BC], f16, tag="rh16", name=f"rh_{t}")
            nc.vector.tensor_mul(rh16[:, :], r[:, :], h_prev)
            acc_mm(ps_h, sl, whh, rh16)
            # u in fp16: together with fp16 s it puts the critical-path STT
            # (t1 = dt*u*s) in the DVE 2x_1p perf mode
            u = work.tile([DIM_OUT, BC], f16, tag="u", name=f"u_{t}")
            nc.scalar.activation(u[:, :], ps_h[:, sl], AF.Tanh)

            # t1 = dt*u*s  (fp16, feeds next step's matmuls; ON critical path)
            t116 = work.tile([DIM_OUT, BC], f16, tag="t116", name=f"t1_{t}")
            nc.vector.scalar_tensor_tensor(
                t116[:, :], u[:, :], DT, sz[:, :], ALU.mult, ALU.mult
            )
            # q = 1 - dt*s ; pre = q*h (f32 + f16 copy); h' = pre + t1 (f32)
            q = work.tile([DIM_OUT, BC], f32, tag="q", name=f"q_{t}")
            nc.vector.tensor_scalar(q[:, :], sz[:, :], -DT, 1.0, ALU.mult, ALU.add)
            pre32 = work.tile([DIM_OUT, BC], f32, tag="pre32", name=f"pre32_{t}")
            nc.vector.tensor_mul(pre32[:, :], q[:, :], h_prev)
            pre16 = work.tile([DIM_OUT, BC], f16, tag="pre16", name=f"pre16_{t}")
            nc.vector.tensor_copy(pre16[:, :], pre32[:, :])
            hnew = hbuf[:, (t % TGROUP) * BC : (t % TGROUP + 1) * BC]
            nc.vector.tensor_add(hnew, pre32[:, :], t116[:, :])
            h_prev = hnew

            if t % TGROUP == TGROUP - 1:
                stg = spool.tile([DIM_OUT, TGROUP * BC], f32, tag="stg", name=f"st_{t}")
                nc.vector.transpose(stg[:, :], hbuf[:, :])
                # stg[32i+b, 32s+c] = h_{t0+s}[32i+c, b]; DMA one feature
                # block i at a time (DMA APs are limited to 3 dims).
                for i in range(DIM_OUT // 32):
                    dst = out_d.ap()[
                        0:BC, t - (TGROUP - 1) : t + 1, 32 * i : 32 * (i + 1)
                    ]
                    nc.sync.dma_start(dst, stg[32 * i : 32 * (i + 1), :])

    nc.compile()
    return nc


def _host_prep(X, W_hr, b_hr, W_hz, b_hz, W_hh, b_hh, h0, t_steps=T):
    f = np.float32
    X = np.asarray(X, f)[:t_steps]
    W_hr, W_hz, W_hh = (np.asarray(w, f) for w in (W_hr, W_hz, W_hh))
    b_hr, b_hz, b_hh = (np.asarray(b, f) for b in (b_hr, b_hz, b_hh))
    h0 = np.asarray(h0, f).reshape(1, DIM_OUT)

    XT = np.ascontiguousarray(np.transpose(X, (2, 0, 1)))  # [64, T, B]
    weights = {
        "wrh": W_hr[:DIM_OUT].astype(np.float16),
        "wzh": (-W_hz[:DIM_OUT]).astype(np.float16),
        "whh": W_hh[:DIM_OUT].astype(np.float16),
    }
    for g, W, b, sgn in (
        ("r", W_hr, b_hr, 1.0),
        ("z", W_hz, b_hz, -1.0),
        ("h", W_hh, b_hh, 1.0),
    ):
        wxb = sgn * np.vstack([W[DIM_OUT:], b[None, :]])  # [65, 128] f32
        hi = wxb.astype(np.float16)
        lo = (wxb - hi.astype(f)).astype(np.float16)
        weights[f"w{g}x_hi"] = np.ascontiguousarray(hi)
        weights[f"w{g}x_lo"] = np.ascontiguousarray(lo)
    weights = {k: np.ascontiguousarray(v) for k, v in weights.items()}
    h0T = np.ascontiguousarray(np.broadcast_to(h0.T, (DIM_OUT, BC)))

    in_maps = []
    for ci in range(NCORES):
        xc = XT[:, :, ci * BC : (ci + 1) * BC].reshape(DIM_IN, t_steps * BC)
        xa = np.ascontiguousarray(
            np.vstack([xc, np.ones((1, t_steps * BC), f)]).astype(np.float16)
        )
        m = {"xa": xa, "h0": h0T}
        m.update(weights)
        in_maps.append(m)
    return in_maps


def run(inputs, trace=False, t_steps=T, tmpdir=None):
    from concourse import bass_utils

    in_maps = _host_prep(**inputs, t_steps=t_steps)
    nc = _build_nc(t_steps)
    res = bass_utils.run_bass_kernel_spmd(
        nc, in_maps, core_ids=list(range(NCORES)), trace=trace, tmpdir=tmpdir
    )
    out = np.concatenate([res.results[i]["out"] for i in range(NCORES)], axis=0)
    return out, res


def kernel(**inputs) -> np.ndarray:
    out, _ = run(inputs, trace=False)
    return out
